# revision 29
# baseline (speedup 1.0000x reference)
"""Trainium2 Bass kernel for AllAtomEnergyBranch (3-layer MLP over broadcast concat).

Math (per batch b, position n, edge e):
    out[b,n,e,0] = W3^T relu(W2^T relu(Wh^T h[b,n] + We^T e_feat[e] + b1) + b2) + b3

Sharding: data-parallel over B (8 batches -> 8 NeuronCores), weights replicated.
Each core computes its [64, 256] output slice independently; no collectives.

Per-core dataflow (k-major layouts so the PE contracts over partitions):
  - epT [512k, 256e] = (We_aug.T @ eT_aug) with b1 folded in via an augmented
    ones-row (K=65), stored bf16 in SBUF.
  - hpT [512k, 64n]  = Wh.T @ hT, stored f32 in SBUF.
  - per block of 2 n's (32 blocks):
      X^T[kt] [128,512] = relu(epT[kt] + hpT[kt][:,n])   (DVE tensor_scalar add+max)
      psumY[jt] = sum_kt W2[kt,jt].T @ X^T[kt]           (PE, bf16)
      Y^T[jt] = relu(psumY[jt] + b2[jt])                 (ACT, per-partition bias)
      Z^T[jt] = Y^T[jt] * w3[jt]  (signed, post-relu)    (DVE per-partition mul)
      ssum    = sum_jt Z^T[jt]    (3 adds)               (DVE)
      psumO   = ones.T @ ssum     (partition reduce)     (PE, single matmul)
      out_blk = psumO + b3                               (ACT Identity + bias)
"""

import numpy as np
import ml_dtypes

import concourse.bass as bass
import concourse.mybir as mybir
from concourse import bacc
from concourse.bass import ts
from concourse.tile import TileContext
from concourse.bass_utils import run_bass_kernel_spmd

BF16 = mybir.dt.bfloat16
F32 = mybir.dt.float32

B, N, H = 8, 64, 256
NE, E = 256, 64
HID, OUT = 512, 1
KT = HID // 128   # 4 k-tiles of layer-1 output / layer-2 contraction
JT = HID // 128   # 4 j-tiles of layer-2 output / layer-3 contraction
HT = H // 128     # 2 h-tiles of layer-1 contraction
NBLK = N // 2     # blocks of 2 n-values -> 512 moving columns per matmul


def build(nc, repeat=1, dyn_repeat=None, y_bufs=6, x_bufs=3, yp_bufs=3,
          mm3_dve=1, o_bufs=2, ones_ct=0):
    """Build the per-core graph. All 8 cores run this same program.

    repeat / dyn_repeat: repeat the whole computation inside the NEFF
    (python-unrolled / For_i hardware loop) — used only for benchmarking.
    """
    ht_d = nc.declare_dram_parameter("ht", [HT, 128, N], BF16, isOutput=False)
    wh_d = nc.declare_dram_parameter("wh", [HT, 128, HID], BF16, isOutput=False)
    we_d = nc.declare_dram_parameter("we", [E + 1, HID], BF16, isOutput=False)
    et_d = nc.declare_dram_parameter("et", [E + 1, NE], BF16, isOutput=False)
    w2_d = nc.declare_dram_parameter("w2", [KT, 128, HID], BF16, isOutput=False)
    b2_d = nc.declare_dram_parameter("b2", [128, JT], F32, isOutput=False)
    w3_d = nc.declare_dram_parameter("w3", [128, JT], BF16, isOutput=False)
    w3f_d = nc.declare_dram_parameter("w3f", [128, JT], F32, isOutput=False)
    b3_d = nc.declare_dram_parameter("b3", [1, 1], F32, isOutput=False)
    out_d = nc.declare_dram_parameter("out", [NBLK, 512], F32, isOutput=True)

    relu = mybir.ActivationFunctionType.Relu
    ident = mybir.ActivationFunctionType.Identity
    add = mybir.AluOpType.add
    mult = mybir.AluOpType.mult
    amax = mybir.AluOpType.max

    with TileContext(nc) as tc:
        with (
            tc.tile_pool(name="const", bufs=1) as cpool,
            tc.tile_pool(name="xp", bufs=x_bufs) as xpool,
            tc.tile_pool(name="yp", bufs=yp_bufs) as ypool,
            tc.tile_pool(name="op", bufs=4) as opool,
            tc.tile_pool(name="psY", bufs=y_bufs, space="PSUM") as y_ps,
            tc.tile_pool(name="psO", bufs=o_bufs, space="PSUM") as o_ps,
        ):
            # ---- load weights / inputs into SBUF ----
            # Order matters: everything the preamble matmuls need (ht/we/et/wh)
            # goes first so the PE can start while W2 is still in flight.
            ht_t = []
            for h in range(HT):
                t = cpool.tile([128, N], BF16, tag=f"ht{h}", name=f"ht{h}")
                nc.sync.dma_start(out=t[:], in_=ht_d[h])
                ht_t.append(t)
            we_t = cpool.tile([E + 1, HID], BF16, tag="we")
            nc.sync.dma_start(out=we_t[:], in_=we_d[:])
            et_t = cpool.tile([E + 1, NE], BF16, tag="et")
            nc.sync.dma_start(out=et_t[:], in_=et_d[:])
            wh_t = []
            for h in range(HT):
                t = cpool.tile([128, HID], BF16, tag=f"wh{h}", name=f"wh{h}")
                nc.sync.dma_start(out=t[:], in_=wh_d[h])
                wh_t.append(t)
            b2_t = cpool.tile([128, JT], F32, tag="b2")
            nc.sync.dma_start(out=b2_t[:], in_=b2_d[:])
            w3_t = cpool.tile([128, JT], BF16, tag="w3")
            nc.sync.dma_start(out=w3_t[:], in_=w3_d[:])
            w3f_t = cpool.tile([128, JT], F32, tag="w3f")
            nc.sync.dma_start(out=w3f_t[:], in_=w3f_d[:])
            ones_t = cpool.tile([128, 1], BF16, tag="ones")
            nc.vector.memset(ones_t[:], 1.0)
            b3_t = cpool.tile([1, 1], F32, tag="b3")
            nc.sync.dma_start(out=b3_t[:], in_=b3_d[:])
            w2_t = []
            for k in range(KT):
                t = cpool.tile([128, HID], BF16, tag=f"w2{k}", name=f"w2{k}")
                nc.sync.dma_start(out=t[:], in_=w2_d[k])
                w2_t.append(t)

            ep_t = [cpool.tile([128, NE], BF16, tag=f"ep{k}", name=f"ep{k}") for k in range(KT)]
            hp_t = [cpool.tile([128, N], F32, tag=f"hp{k}", name=f"hp{k}") for k in range(KT)]

            def body():
                # ---- preamble: epT (with b1 via aug row) and hpT ----
                for k in range(KT):
                    ps = y_ps.tile([128, NE], F32, tag="Y", name="psE")
                    nc.tensor.matmul(
                        ps[:], we_t[:, ts(k, 128)], et_t[:], start=True, stop=True
                    )
                    nc.vector.tensor_copy(out=ep_t[k][:], in_=ps[:])
                for k in range(KT):
                    ps = y_ps.tile([128, N], F32, tag="Y", name="psH")
                    for h in range(HT):
                        nc.tensor.matmul(
                            ps[:],
                            wh_t[h][:, ts(k, 128)],
                            ht_t[h][:],
                            start=(h == 0),
                            stop=(h == HT - 1),
                        )
                    nc.vector.tensor_copy(out=hp_t[k][:], in_=ps[:])

                # ---- main loop over blocks of 2 n-values ----
                ssum_q = []   # (blk, ssum) pending partition-reduce (ones_ct)
                for blk in range(NBLK):
                    xt = []
                    for k in range(KT):
                        t = xpool.tile([128, 512], BF16, tag=f"x{k}", name=f"x{k}")
                        for j in range(2):
                            n = 2 * blk + j
                            nc.vector.tensor_scalar(
                                out=t[:, ts(j, NE)],
                                in0=ep_t[k][:],
                                scalar1=hp_t[k][:, n : n + 1],
                                scalar2=0.0,
                                op0=add,
                                op1=amax,
                            )
                        xt.append(t)

                    pso = None if ones_ct else o_ps.tile([1, 512], F32, tag="po")
                    zts = []
                    # last block: PE-native W3 matmuls (interleave with its own
                    # mm2s) so the kernel tail doesn't wait on the DVE fold
                    blk_dve = mm3_dve and blk < NBLK - 1
                    for j in range(JT):
                        psy = y_ps.tile([128, 512], F32, tag="Y")
                        for k in range(KT):
                            nc.tensor.matmul(
                                psy[:],
                                w2_t[k][:, ts(j, 128)],
                                xt[k][:],
                                start=(k == 0),
                                stop=(k == KT - 1),
                            )
                        yt = ypool.tile([128, 512], BF16, tag=f"y{j}", name=f"y{j}")
                        nc.scalar.activation(
                            out=yt[:],
                            in_=psy[:],
                            func=relu,
                            bias=b2_t[:, j : j + 1],
                            scale=1.0,
                        )
                        if blk_dve:
                            # fold signed w3 now (y >= 0 post-relu, so a plain
                            # per-partition multiply is exact w3*relu(.))
                            zt = ypool.tile([128, 512], BF16,
                                            tag=f"z{j}", name=f"z{j}")
                            nc.vector.tensor_scalar(
                                out=zt[:],
                                in0=yt[:],
                                scalar1=w3f_t[:, j : j + 1],
                                scalar2=None,
                                op0=mult,
                            )
                            zts.append(zt)
                        else:
                            nc.tensor.matmul(
                                pso[:],
                                w3_t[:, j : j + 1],
                                yt[:],
                                start=(j == 0),
                                stop=(j == JT - 1),
                            )
                    if blk_dve:
                        s01 = ypool.tile([128, 512], BF16, tag="s01", name="s01")
                        nc.vector.tensor_add(out=s01[:], in0=zts[0][:], in1=zts[1][:])
                        s23 = ypool.tile([128, 512], BF16, tag="s23", name="s23")
                        nc.vector.tensor_add(out=s23[:], in0=zts[2][:], in1=zts[3][:])
                        ssum = ypool.tile([128, 512], BF16, tag="ss", name="ss",
                                          bufs=6 if ones_ct else None)
                        nc.vector.tensor_add(out=ssum[:], in0=s01[:], in1=s23[:])
                        if ones_ct:
                            # batch 4 blocks; col-tiled ones-matmuls run in
                            # different array column groups -> concurrent
                            ssum_q.append((blk, ssum))
                            if len(ssum_q) == 4:
                                pso4 = o_ps.tile([128, 512], F32, tag="po4",
                                                 name="po4")
                                for bi, (b_, ss_) in enumerate(ssum_q):
                                    nc.tensor.matmul(
                                        pso4[32 * bi : 32 * bi + 1, :],
                                        ones_t[:],
                                        ss_[:],
                                        start=True,
                                        stop=True,
                                        tile_position=(0, 32 * bi),
                                    )
                                for bi, (b_, ss_) in enumerate(ssum_q):
                                    ot = opool.tile([1, 512], F32, tag="o")
                                    nc.scalar.activation(
                                        out=ot[:],
                                        in_=pso4[32 * bi : 32 * bi + 1, :],
                                        func=ident,
                                        bias=b3_t[0:1, 0:1],
                                        scale=1.0,
                                    )
                                    nc.sync.dma_start(
                                        out=out_d[b_ : b_ + 1, :], in_=ot[:])
                                ssum_q = []
                            continue
                        nc.tensor.matmul(
                            pso[:], ones_t[:], ssum[:], start=True, stop=True
                        )
                    ot = opool.tile([1, 512], F32, tag="o")
                    nc.scalar.activation(
                        out=ot[:],
                        in_=pso[:],
                        func=ident,
                        bias=b3_t[0:1, 0:1],
                        scale=1.0,
                    )
                    nc.sync.dma_start(out=out_d[blk : blk + 1, :], in_=ot[:])

            if dyn_repeat is not None:
                hint = (mybir.EngineType.PE, mybir.EngineType.DVE,
                        mybir.EngineType.Activation)
                with tc.For_i(0, dyn_repeat, 1, hint_engines=hint):
                    body()
            else:
                for _rep in range(repeat):
                    body()
    return nc


def make_in_maps(h_all, e_feat, W1, b1, W2, b2, W3, b3):
    bf = ml_dtypes.bfloat16
    Wh = np.ascontiguousarray(W1[:H]).astype(bf).reshape(HT, 128, HID)
    We_aug = np.concatenate([W1[H:], b1[None, :]], axis=0).astype(bf)
    eT_aug = np.concatenate(
        [e_feat.T, np.ones((1, NE), np.float32)], axis=0
    ).astype(bf)
    W2k = W2.astype(bf).reshape(KT, 128, HID)
    b2c = np.ascontiguousarray(b2.reshape(JT, 128).T).astype(np.float32)
    W3c = np.ascontiguousarray(W3.reshape(JT, 128).T).astype(bf)
    W3f = np.ascontiguousarray(W3.reshape(JT, 128).T).astype(np.float32)
    b3c = np.asarray(b3, np.float32).reshape(1, 1)
    shared = {
        "wh": Wh, "we": We_aug, "et": eT_aug, "w2": W2k,
        "b2": b2c, "w3": W3c, "w3f": W3f, "b3": b3c,
    }
    in_maps = []
    for b in range(B):
        hT = np.ascontiguousarray(h_all[b].T).astype(bf).reshape(HT, 128, N)
        in_maps.append({"ht": hT, **shared})
    return in_maps


_nc_cache = {}


def _get_nc():
    if "nc" not in _nc_cache:
        nc = bacc.Bacc("TRN2", target_bir_lowering=False, debug=False, num_devices=B)
        build(nc)
        nc.compile()
        _nc_cache["nc"] = nc
    return _nc_cache["nc"]


def kernel(h_all, e_feat, W1, b1, W2, b2, W3, b3):
    h_all = np.asarray(h_all, np.float32)
    e_feat = np.asarray(e_feat, np.float32)
    W1 = np.asarray(W1, np.float32)
    b1 = np.asarray(b1, np.float32)
    W2 = np.asarray(W2, np.float32)
    b2 = np.asarray(b2, np.float32)
    W3 = np.asarray(W3, np.float32)
    b3 = np.asarray(b3, np.float32)

    nc = _get_nc()
    in_maps = make_in_maps(h_all, e_feat, W1, b1, W2, b2, W3, b3)
    res = run_bass_kernel_spmd(nc, in_maps, core_ids=list(range(B)))
    out = np.stack([res.results[i]["out"].reshape(N, NE, OUT) for i in range(B)])
    return out.astype(np.float32)


# revision 33
# speedup vs baseline: 1.0060x; 1.0060x over previous
"""Trainium2 Bass kernel for AllAtomEnergyBranch (3-layer MLP over broadcast concat).

Math (per batch b, position n, edge e):
    out[b,n,e,0] = W3^T relu(W2^T relu(Wh^T h[b,n] + We^T e_feat[e] + b1) + b2) + b3

Sharding: data-parallel over B (8 batches -> 8 NeuronCores), weights replicated.
Each core computes its [64, 256] output slice independently; no collectives.

Per-core dataflow (k-major layouts so the PE contracts over partitions):
  - epT [512k, 256e] = (We_aug.T @ eT_aug) with b1 folded in via an augmented
    ones-row (K=65), stored bf16 in SBUF.
  - hpT [512k, 64n]  = Wh.T @ hT, stored f32 in SBUF.
  - per block of 2 n's (32 blocks):
      X^T[kt] [128,512] = relu(epT[kt] + hpT[kt][:,n])   (DVE tensor_scalar add+max)
      psumY[jt] = sum_kt W2[kt,jt].T @ X^T[kt]           (PE, bf16)
      Y^T[jt] = relu(psumY[jt] + b2[jt])                 (ACT, per-partition bias)
      Z^T[jt] = Y^T[jt] * w3[jt]  (signed, post-relu)    (DVE per-partition mul)
      ssum    = sum_jt Z^T[jt]    (3 adds)               (DVE)
      psumO   = ones.T @ ssum     (partition reduce)     (PE, single matmul)
      out_blk = psumO + b3                               (ACT Identity + bias)
"""

import numpy as np
import ml_dtypes

import concourse.bass as bass
import concourse.mybir as mybir
from concourse import bacc
from concourse.bass import ts
from concourse.tile import TileContext
from concourse.bass_utils import run_bass_kernel_spmd

BF16 = mybir.dt.bfloat16
F32 = mybir.dt.float32

B, N, H = 8, 64, 256
NE, E = 256, 64
HID, OUT = 512, 1
KT = HID // 128   # 4 k-tiles of layer-1 output / layer-2 contraction
JT = HID // 128   # 4 j-tiles of layer-2 output / layer-3 contraction
HT = H // 128     # 2 h-tiles of layer-1 contraction
NBLK = N // 2     # blocks of 2 n-values -> 512 moving columns per matmul


def build(nc, repeat=1, dyn_repeat=None, y_bufs=6, x_bufs=3, yp_bufs=3,
          mm3_dve=1, o_bufs=2, ones_ct=0):
    """Build the per-core graph. All 8 cores run this same program.

    repeat / dyn_repeat: repeat the whole computation inside the NEFF
    (python-unrolled / For_i hardware loop) — used only for benchmarking.
    """
    ht_d = nc.declare_dram_parameter("ht", [HT, 128, N], BF16, isOutput=False)
    wh_d = nc.declare_dram_parameter("wh", [HT, 128, HID], BF16, isOutput=False)
    we_d = nc.declare_dram_parameter("we", [E + 1, HID], BF16, isOutput=False)
    et_d = nc.declare_dram_parameter("et", [E + 1, NE], BF16, isOutput=False)
    w2_d = nc.declare_dram_parameter("w2", [KT, 128, HID], BF16, isOutput=False)
    b2_d = nc.declare_dram_parameter("b2", [128, JT], F32, isOutput=False)
    w3_d = nc.declare_dram_parameter("w3", [128, JT], BF16, isOutput=False)
    w3f_d = nc.declare_dram_parameter("w3f", [128, JT], F32, isOutput=False)
    b3_d = nc.declare_dram_parameter("b3", [1, 1], F32, isOutput=False)
    out_d = nc.declare_dram_parameter("out", [NBLK, 512], F32, isOutput=True)

    relu = mybir.ActivationFunctionType.Relu
    ident = mybir.ActivationFunctionType.Identity
    add = mybir.AluOpType.add
    mult = mybir.AluOpType.mult
    amax = mybir.AluOpType.max

    with TileContext(nc) as tc:
        with (
            tc.tile_pool(name="const", bufs=1) as cpool,
            tc.tile_pool(name="xp", bufs=x_bufs) as xpool,
            tc.tile_pool(name="yp", bufs=yp_bufs) as ypool,
            tc.tile_pool(name="op", bufs=4) as opool,
            tc.tile_pool(name="psY", bufs=y_bufs, space="PSUM") as y_ps,
            tc.tile_pool(name="psO", bufs=o_bufs, space="PSUM") as o_ps,
        ):
            # ---- load weights / inputs into SBUF ----
            # Order matters: everything the preamble matmuls need (ht/we/et/wh)
            # goes first so the PE can start while W2 is still in flight.
            ht_t = []
            for h in range(HT):
                t = cpool.tile([128, N], BF16, tag=f"ht{h}", name=f"ht{h}")
                nc.sync.dma_start(out=t[:], in_=ht_d[h])
                ht_t.append(t)
            we_t = cpool.tile([E + 1, HID], BF16, tag="we")
            nc.sync.dma_start(out=we_t[:], in_=we_d[:])
            et_t = cpool.tile([E + 1, NE], BF16, tag="et")
            nc.sync.dma_start(out=et_t[:], in_=et_d[:])
            wh_t = []
            for h in range(HT):
                t = cpool.tile([128, HID], BF16, tag=f"wh{h}", name=f"wh{h}")
                nc.sync.dma_start(out=t[:], in_=wh_d[h])
                wh_t.append(t)
            b2_t = cpool.tile([128, JT], F32, tag="b2")
            nc.sync.dma_start(out=b2_t[:], in_=b2_d[:])
            w3_t = cpool.tile([128, JT], BF16, tag="w3")
            nc.sync.dma_start(out=w3_t[:], in_=w3_d[:])
            w3f_t = cpool.tile([128, JT], F32, tag="w3f")
            nc.sync.dma_start(out=w3f_t[:], in_=w3f_d[:])
            ones_t = cpool.tile([128, 1], BF16, tag="ones")
            nc.vector.memset(ones_t[:], 1.0)
            b3_t = cpool.tile([1, 1], F32, tag="b3")
            nc.sync.dma_start(out=b3_t[:], in_=b3_d[:])
            w2_t = []
            for k in range(KT):
                t = cpool.tile([128, HID], BF16, tag=f"w2{k}", name=f"w2{k}")
                nc.sync.dma_start(out=t[:], in_=w2_d[k])
                w2_t.append(t)

            ep_t = [cpool.tile([128, NE], BF16, tag=f"ep{k}", name=f"ep{k}") for k in range(KT)]
            hp_t = [cpool.tile([128, N], F32, tag=f"hp{k}", name=f"hp{k}") for k in range(KT)]

            def body():
                # ---- preamble: epT (with b1 via aug row) and hpT ----
                for k in range(KT):
                    ps = y_ps.tile([128, NE], F32, tag="Y", name="psE")
                    nc.tensor.matmul(
                        ps[:], we_t[:, ts(k, 128)], et_t[:], start=True, stop=True
                    )
                    nc.vector.tensor_copy(out=ep_t[k][:], in_=ps[:])
                for k in range(KT):
                    ps = y_ps.tile([128, N], F32, tag="Y", name="psH")
                    for h in range(HT):
                        nc.tensor.matmul(
                            ps[:],
                            wh_t[h][:, ts(k, 128)],
                            ht_t[h][:],
                            start=(h == 0),
                            stop=(h == HT - 1),
                        )
                    nc.vector.tensor_copy(out=hp_t[k][:], in_=ps[:])

                # ---- main loop over blocks of 2 n-values ----
                ssum_q = []   # (blk, ssum) pending partition-reduce (ones_ct)
                for blk in range(NBLK):
                    xt = []
                    for k in range(KT):
                        t = xpool.tile([128, 512], BF16, tag=f"x{k}", name=f"x{k}")
                        for j in range(2):
                            n = 2 * blk + j
                            nc.vector.tensor_scalar(
                                out=t[:, ts(j, NE)],
                                in0=ep_t[k][:],
                                scalar1=hp_t[k][:, n : n + 1],
                                scalar2=0.0,
                                op0=add,
                                op1=amax,
                            )
                        xt.append(t)

                    pso = None if ones_ct else o_ps.tile([1, 512], F32, tag="po")
                    zts = []
                    # last block: PE-native W3 matmuls (interleave with its own
                    # mm2s) so the kernel tail doesn't wait on the DVE fold
                    blk_dve = mm3_dve and blk < NBLK - 1
                    for j in range(JT):
                        psy = y_ps.tile([128, 512], F32, tag="Y")
                        for k in range(KT):
                            nc.tensor.matmul(
                                psy[:],
                                w2_t[k][:, ts(j, 128)],
                                xt[k][:],
                                start=(k == 0),
                                stop=(k == KT - 1),
                            )
                        yt = ypool.tile([128, 512], BF16, tag=f"y{j}", name=f"y{j}")
                        nc.scalar.activation(
                            out=yt[:],
                            in_=psy[:],
                            func=relu,
                            bias=b2_t[:, j : j + 1],
                            scale=1.0,
                        )
                        if blk_dve:
                            # fold signed w3 now (y >= 0 post-relu, so a plain
                            # per-partition multiply is exact w3*relu(.))
                            zt = ypool.tile([128, 512], BF16,
                                            tag=f"z{j}", name=f"z{j}")
                            nc.vector.tensor_scalar(
                                out=zt[:],
                                in0=yt[:],
                                scalar1=w3f_t[:, j : j + 1],
                                scalar2=None,
                                op0=mult,
                            )
                            zts.append(zt)
                        else:
                            nc.tensor.matmul(
                                pso[:],
                                w3_t[:, j : j + 1],
                                yt[:],
                                start=(j == 0),
                                stop=(j == JT - 1),
                            )
                    if blk_dve:
                        s01 = ypool.tile([128, 512], BF16, tag="s01", name="s01")
                        nc.vector.tensor_add(out=s01[:], in0=zts[0][:], in1=zts[1][:])
                        s23 = ypool.tile([128, 512], BF16, tag="s23", name="s23")
                        nc.vector.tensor_add(out=s23[:], in0=zts[2][:], in1=zts[3][:])
                        ssum = ypool.tile([128, 512], BF16, tag="ss", name="ss",
                                          bufs=6 if ones_ct else None)
                        nc.vector.tensor_add(out=ssum[:], in0=s01[:], in1=s23[:])
                        if ones_ct:
                            # batch 4 blocks; col-tiled ones-matmuls run in
                            # different array column groups -> concurrent
                            ssum_q.append((blk, ssum))
                            if len(ssum_q) == 4:
                                pso4 = o_ps.tile([128, 512], F32, tag="po4",
                                                 name="po4")
                                for bi, (b_, ss_) in enumerate(ssum_q):
                                    nc.tensor.matmul(
                                        pso4[32 * bi : 32 * bi + 1, :],
                                        ones_t[:],
                                        ss_[:],
                                        start=True,
                                        stop=True,
                                        tile_position=(0, 32 * bi),
                                    )
                                for bi, (b_, ss_) in enumerate(ssum_q):
                                    ot = opool.tile([1, 512], F32, tag="o")
                                    nc.scalar.activation(
                                        out=ot[:],
                                        in_=pso4[32 * bi : 32 * bi + 1, :],
                                        func=ident,
                                        bias=b3_t[0:1, 0:1],
                                        scale=1.0,
                                    )
                                    nc.sync.dma_start(
                                        out=out_d[b_ : b_ + 1, :], in_=ot[:])
                                ssum_q = []
                            continue
                        nc.tensor.matmul(
                            pso[:], ones_t[:], ssum[:], start=True, stop=True
                        )
                    ot = opool.tile([1, 512], F32, tag="o")
                    nc.scalar.activation(
                        out=ot[:],
                        in_=pso[:],
                        func=ident,
                        bias=b3_t[0:1, 0:1],
                        scale=1.0,
                    )
                    nc.sync.dma_start(out=out_d[blk : blk + 1, :], in_=ot[:])

            if dyn_repeat is not None:
                hint = (mybir.EngineType.PE, mybir.EngineType.DVE,
                        mybir.EngineType.Activation)
                with tc.For_i(0, dyn_repeat, 1, hint_engines=hint):
                    body()
            else:
                for _rep in range(repeat):
                    body()
    return nc


def make_in_maps(h_all, e_feat, W1, b1, W2, b2, W3, b3):
    bf = ml_dtypes.bfloat16
    Wh = np.ascontiguousarray(W1[:H]).astype(bf).reshape(HT, 128, HID)
    We_aug = np.concatenate([W1[H:], b1[None, :]], axis=0).astype(bf)
    eT_aug = np.concatenate(
        [e_feat.T, np.ones((1, NE), np.float32)], axis=0
    ).astype(bf)
    W2k = W2.astype(bf).reshape(KT, 128, HID)
    b2c = np.ascontiguousarray(b2.reshape(JT, 128).T).astype(np.float32)
    W3c = np.ascontiguousarray(W3.reshape(JT, 128).T).astype(bf)
    W3f = np.ascontiguousarray(W3.reshape(JT, 128).T).astype(np.float32)
    b3c = np.asarray(b3, np.float32).reshape(1, 1)
    shared = {
        "wh": Wh, "we": We_aug, "et": eT_aug, "w2": W2k,
        "b2": b2c, "w3": W3c, "w3f": W3f, "b3": b3c,
    }
    in_maps = []
    for b in range(B):
        hT = np.ascontiguousarray(h_all[b].T).astype(bf).reshape(HT, 128, N)
        in_maps.append({"ht": hT, **shared})
    return in_maps


_nc_cache = {}


def _get_nc():
    if "nc" not in _nc_cache:
        nc = bacc.Bacc("TRN2", target_bir_lowering=False, debug=False, num_devices=B)
        build(nc)
        nc.compile()
        _nc_cache["nc"] = nc
    return _nc_cache["nc"]


def kernel(h_all, e_feat, W1, b1, W2, b2, W3, b3):
    h_all = np.asarray(h_all, np.float32)
    e_feat = np.asarray(e_feat, np.float32)
    W1 = np.asarray(W1, np.float32)
    b1 = np.asarray(b1, np.float32)
    W2 = np.asarray(W2, np.float32)
    b2 = np.asarray(b2, np.float32)
    W3 = np.asarray(W3, np.float32)
    b3 = np.asarray(b3, np.float32)

    nc = _get_nc()
    in_maps = make_in_maps(h_all, e_feat, W1, b1, W2, b2, W3, b3)
    res = run_bass_kernel_spmd(nc, in_maps, core_ids=list(range(B)))
    out = np.stack([res.results[i]["out"].reshape(N, NE, OUT) for i in range(B)])
    return out.astype(np.float32)


# revision 36
# speedup vs baseline: 1.1165x; 1.1098x over previous
"""Trainium2 Bass kernel for AllAtomEnergyBranch (3-layer MLP over broadcast concat).

Math (per batch b, position n, edge e):
    out[b,n,e,0] = W3^T relu(W2^T relu(Wh^T h[b,n] + We^T e_feat[e] + b1) + b2) + b3

Sharding: data-parallel over B (8 batches -> 8 NeuronCores), weights replicated.
Each core computes its [64, 256] output slice independently; no collectives.

Per-core dataflow (k-major layouts so the PE contracts over partitions):
  - epT [512k, 256e] = (We_aug.T @ eT_aug) with b1 folded in via an augmented
    ones-row (K=65), stored bf16 in SBUF.
  - hpT [512k, 64n]  = Wh.T @ hT, stored f32 in SBUF.
  - per block of 2 n's (32 blocks):
      X^T[kt] [128,512] = relu(epT[kt] + hpT[kt][:,n])   (DVE tensor_scalar add+max)
      psumY[jt] = sum_kt W2[kt,jt].T @ X^T[kt]           (PE, bf16)
      Y^T[jt] = relu(psumY[jt] + b2[jt])                 (ACT, per-partition bias)
      Z^T[jt] = Y^T[jt] * w3[jt]  (signed, post-relu)    (DVE per-partition mul)
      ssum    = sum_jt Z^T[jt]    (3 adds)               (DVE)
      psumO   = ones.T @ ssum     (partition reduce)     (PE, single matmul)
      out_blk = psumO + b3                               (ACT Identity + bias)
"""

import numpy as np
import ml_dtypes

import concourse.bass as bass
import concourse.mybir as mybir
from concourse import bacc
from concourse.bass import ts
from concourse.tile import TileContext
from concourse.bass_utils import run_bass_kernel_spmd

BF16 = mybir.dt.bfloat16
F32 = mybir.dt.float32

B, N, H = 8, 64, 256
NE, E = 256, 64
HID, OUT = 512, 1
KT = HID // 128   # 4 k-tiles of layer-1 output / layer-2 contraction
JT = HID // 128   # 4 j-tiles of layer-2 output / layer-3 contraction
HT = H // 128     # 2 h-tiles of layer-1 contraction
NBLK = N // 2     # blocks of 2 n-values -> 512 moving columns per matmul


def build(nc, repeat=1, dyn_repeat=None, y_bufs=6, x_bufs=3, yp_bufs=3,
          mm3_dve=1, o_bufs=2, ones_ct=0):
    """Build the per-core graph. All 8 cores run this same program.

    repeat / dyn_repeat: repeat the whole computation inside the NEFF
    (python-unrolled / For_i hardware loop) — used only for benchmarking.
    """
    ht_d = nc.declare_dram_parameter("ht", [HT, 128, N], BF16, isOutput=False)
    wh_d = nc.declare_dram_parameter("wh", [HT, 128, HID], BF16, isOutput=False)
    we_d = nc.declare_dram_parameter("we", [E + 1, HID], BF16, isOutput=False)
    et_d = nc.declare_dram_parameter("et", [E + 1, NE], BF16, isOutput=False)
    w2_d = nc.declare_dram_parameter("w2", [KT, 128, HID], BF16, isOutput=False)
    b2_d = nc.declare_dram_parameter("b2", [128, JT], F32, isOutput=False)
    w3_d = nc.declare_dram_parameter("w3", [128, JT], BF16, isOutput=False)
    w3f_d = nc.declare_dram_parameter("w3f", [128, JT], F32, isOutput=False)
    b3_d = nc.declare_dram_parameter("b3", [1, 1], F32, isOutput=False)
    out_d = nc.declare_dram_parameter("out", [NBLK, 512], F32, isOutput=True)

    relu = mybir.ActivationFunctionType.Relu
    ident = mybir.ActivationFunctionType.Identity
    add = mybir.AluOpType.add
    mult = mybir.AluOpType.mult
    amax = mybir.AluOpType.max

    with TileContext(nc) as tc:
        with (
            tc.tile_pool(name="const", bufs=1) as cpool,
            tc.tile_pool(name="xp", bufs=x_bufs) as xpool,
            tc.tile_pool(name="yp", bufs=yp_bufs) as ypool,
            tc.tile_pool(name="op", bufs=4) as opool,
            tc.tile_pool(name="psY", bufs=y_bufs, space="PSUM") as y_ps,
            tc.tile_pool(name="psO", bufs=o_bufs, space="PSUM") as o_ps,
        ):
            # ---- load weights / inputs into SBUF ----
            # Order matters: everything the preamble matmuls need (ht/we/et/wh)
            # goes first so the PE can start while W2 is still in flight.
            we_t = cpool.tile([E + 1, HID], BF16, tag="we")
            nc.sync.dma_start(out=we_t[:], in_=we_d[:])
            et_t = cpool.tile([E + 1, NE], BF16, tag="et")
            nc.sync.dma_start(out=et_t[:], in_=et_d[:])
            ht_t = []
            for h in range(HT):
                t = cpool.tile([128, N], BF16, tag=f"ht{h}", name=f"ht{h}")
                nc.sync.dma_start(out=t[:], in_=ht_d[h])
                ht_t.append(t)
            wh_t = []
            for h in range(HT):
                t = cpool.tile([128, HID], BF16, tag=f"wh{h}", name=f"wh{h}")
                nc.sync.dma_start(out=t[:], in_=wh_d[h])
                wh_t.append(t)
            b2_t = cpool.tile([128, JT], F32, tag="b2")
            nc.sync.dma_start(out=b2_t[:], in_=b2_d[:])
            w3_t = cpool.tile([128, JT], BF16, tag="w3")
            nc.sync.dma_start(out=w3_t[:], in_=w3_d[:])
            w3f_t = cpool.tile([128, JT], F32, tag="w3f")
            nc.sync.dma_start(out=w3f_t[:], in_=w3f_d[:])
            ones_t = cpool.tile([128, 1], BF16, tag="ones")
            nc.vector.memset(ones_t[:], 1.0)
            b3_t = cpool.tile([1, 1], F32, tag="b3")
            nc.sync.dma_start(out=b3_t[:], in_=b3_d[:])
            w2_t = []
            for k in range(KT):
                t = cpool.tile([128, HID], BF16, tag=f"w2{k}", name=f"w2{k}")
                nc.sync.dma_start(out=t[:], in_=w2_d[k])
                w2_t.append(t)

            ep_t = [cpool.tile([128, NE], BF16, tag=f"ep{k}", name=f"ep{k}") for k in range(KT)]
            hp_t = [cpool.tile([128, N], F32, tag=f"hp{k}", name=f"hp{k}") for k in range(KT)]

            def body():
                # ---- preamble: epT (with b1 via aug row) and hpT ----
                for k in range(KT):
                    ps = y_ps.tile([128, NE], F32, tag="Y", name="psE")
                    nc.tensor.matmul(
                        ps[:], we_t[:, ts(k, 128)], et_t[:], start=True, stop=True
                    )
                    nc.vector.tensor_copy(out=ep_t[k][:], in_=ps[:])
                for k in range(KT):
                    ps = y_ps.tile([128, N], F32, tag="Y", name="psH")
                    for h in range(HT):
                        nc.tensor.matmul(
                            ps[:],
                            wh_t[h][:, ts(k, 128)],
                            ht_t[h][:],
                            start=(h == 0),
                            stop=(h == HT - 1),
                        )
                    nc.vector.tensor_copy(out=hp_t[k][:], in_=ps[:])

                # ---- main loop over blocks of 2 n-values ----
                ssum_q = []   # (blk, ssum) pending partition-reduce (ones_ct)
                for blk in range(NBLK):
                    xt = []
                    for k in range(KT):
                        t = xpool.tile([128, 512], BF16, tag=f"x{k}", name=f"x{k}")
                        for j in range(2):
                            n = 2 * blk + j
                            nc.vector.tensor_scalar(
                                out=t[:, ts(j, NE)],
                                in0=ep_t[k][:],
                                scalar1=hp_t[k][:, n : n + 1],
                                scalar2=0.0,
                                op0=add,
                                op1=amax,
                            )
                        xt.append(t)

                    pso = None if ones_ct else o_ps.tile([1, 512], F32, tag="po")
                    zts = []
                    # last block: PE-native W3 matmuls (interleave with its own
                    # mm2s) so the kernel tail doesn't wait on the DVE fold
                    blk_dve = mm3_dve and blk < NBLK - 1
                    for j in range(JT):
                        psy = y_ps.tile([128, 512], F32, tag="Y")
                        for k in range(KT):
                            nc.tensor.matmul(
                                psy[:],
                                w2_t[k][:, ts(j, 128)],
                                xt[k][:],
                                start=(k == 0),
                                stop=(k == KT - 1),
                            )
                        yt = ypool.tile([128, 512], BF16, tag=f"y{j}", name=f"y{j}")
                        nc.scalar.activation(
                            out=yt[:],
                            in_=psy[:],
                            func=relu,
                            bias=b2_t[:, j : j + 1],
                            scale=1.0,
                        )
                        if blk_dve:
                            # fold signed w3 now (y >= 0 post-relu, so a plain
                            # per-partition multiply is exact w3*relu(.))
                            zt = ypool.tile([128, 512], BF16,
                                            tag=f"z{j}", name=f"z{j}")
                            nc.vector.tensor_scalar(
                                out=zt[:],
                                in0=yt[:],
                                scalar1=w3f_t[:, j : j + 1],
                                scalar2=None,
                                op0=mult,
                            )
                            zts.append(zt)
                        else:
                            nc.tensor.matmul(
                                pso[:],
                                w3_t[:, j : j + 1],
                                yt[:],
                                start=(j == 0),
                                stop=(j == JT - 1),
                            )
                    if blk_dve:
                        s01 = ypool.tile([128, 512], BF16, tag="s01", name="s01")
                        nc.vector.tensor_add(out=s01[:], in0=zts[0][:], in1=zts[1][:])
                        s23 = ypool.tile([128, 512], BF16, tag="s23", name="s23")
                        nc.vector.tensor_add(out=s23[:], in0=zts[2][:], in1=zts[3][:])
                        ssum = ypool.tile([128, 512], BF16, tag="ss", name="ss",
                                          bufs=6 if ones_ct else None)
                        nc.vector.tensor_add(out=ssum[:], in0=s01[:], in1=s23[:])
                        if ones_ct:
                            # batch 4 blocks; col-tiled ones-matmuls run in
                            # different array column groups -> concurrent
                            ssum_q.append((blk, ssum))
                            if len(ssum_q) == 4:
                                pso4 = o_ps.tile([128, 512], F32, tag="po4",
                                                 name="po4")
                                for bi, (b_, ss_) in enumerate(ssum_q):
                                    nc.tensor.matmul(
                                        pso4[32 * bi : 32 * bi + 1, :],
                                        ones_t[:],
                                        ss_[:],
                                        start=True,
                                        stop=True,
                                        tile_position=(0, 32 * bi),
                                    )
                                for bi, (b_, ss_) in enumerate(ssum_q):
                                    ot = opool.tile([1, 512], F32, tag="o")
                                    nc.scalar.activation(
                                        out=ot[:],
                                        in_=pso4[32 * bi : 32 * bi + 1, :],
                                        func=ident,
                                        bias=b3_t[0:1, 0:1],
                                        scale=1.0,
                                    )
                                    nc.sync.dma_start(
                                        out=out_d[b_ : b_ + 1, :], in_=ot[:])
                                ssum_q = []
                            continue
                        nc.tensor.matmul(
                            pso[:], ones_t[:], ssum[:], start=True, stop=True
                        )
                    ot = opool.tile([1, 512], F32, tag="o")
                    nc.scalar.activation(
                        out=ot[:],
                        in_=pso[:],
                        func=ident,
                        bias=b3_t[0:1, 0:1],
                        scale=1.0,
                    )
                    nc.sync.dma_start(out=out_d[blk : blk + 1, :], in_=ot[:])

            if dyn_repeat is not None:
                hint = (mybir.EngineType.PE, mybir.EngineType.DVE,
                        mybir.EngineType.Activation)
                with tc.For_i(0, dyn_repeat, 1, hint_engines=hint):
                    body()
            else:
                for _rep in range(repeat):
                    body()
    return nc


def make_in_maps(h_all, e_feat, W1, b1, W2, b2, W3, b3):
    bf = ml_dtypes.bfloat16
    Wh = np.ascontiguousarray(W1[:H]).astype(bf).reshape(HT, 128, HID)
    We_aug = np.concatenate([W1[H:], b1[None, :]], axis=0).astype(bf)
    eT_aug = np.concatenate(
        [e_feat.T, np.ones((1, NE), np.float32)], axis=0
    ).astype(bf)
    W2k = W2.astype(bf).reshape(KT, 128, HID)
    b2c = np.ascontiguousarray(b2.reshape(JT, 128).T).astype(np.float32)
    W3c = np.ascontiguousarray(W3.reshape(JT, 128).T).astype(bf)
    W3f = np.ascontiguousarray(W3.reshape(JT, 128).T).astype(np.float32)
    b3c = np.asarray(b3, np.float32).reshape(1, 1)
    shared = {
        "wh": Wh, "we": We_aug, "et": eT_aug, "w2": W2k,
        "b2": b2c, "w3": W3c, "w3f": W3f, "b3": b3c,
    }
    in_maps = []
    for b in range(B):
        hT = np.ascontiguousarray(h_all[b].T).astype(bf).reshape(HT, 128, N)
        in_maps.append({"ht": hT, **shared})
    return in_maps


_nc_cache = {}


def _get_nc():
    if "nc" not in _nc_cache:
        nc = bacc.Bacc("TRN2", target_bir_lowering=False, debug=False, num_devices=B)
        build(nc)
        nc.compile()
        _nc_cache["nc"] = nc
    return _nc_cache["nc"]


def kernel(h_all, e_feat, W1, b1, W2, b2, W3, b3):
    h_all = np.asarray(h_all, np.float32)
    e_feat = np.asarray(e_feat, np.float32)
    W1 = np.asarray(W1, np.float32)
    b1 = np.asarray(b1, np.float32)
    W2 = np.asarray(W2, np.float32)
    b2 = np.asarray(b2, np.float32)
    W3 = np.asarray(W3, np.float32)
    b3 = np.asarray(b3, np.float32)

    nc = _get_nc()
    in_maps = make_in_maps(h_all, e_feat, W1, b1, W2, b2, W3, b3)
    res = run_bass_kernel_spmd(nc, in_maps, core_ids=list(range(B)))
    out = np.stack([res.results[i]["out"].reshape(N, NE, OUT) for i in range(B)])
    return out.astype(np.float32)
